# revision 7
# baseline (speedup 1.0000x reference)
"""Trainium2 Bass kernel v4 for nn_Attention_22050362097758 (edge-biased causal
attention; b=2, n=512, dim=256, heads=8, dim_head=64).

Sharding: core c -> batch c//4, lane l=c%4, query rows i = 4r+l (r=0..127).
Row-interleaving makes causal extents core-uniform (SPMD: one program).

The host packs the causal-prefix edges TWICE in fp8e4 (j-on-partitions for the
sum-of-squares path, d-on-partitions for the bias matmul), one 2 MB DMA per
32-tile chunk -- no on-device transpose. Bias matmul uses the fp8 eT slabs as
PE weights (FWL fast weight load) against bf16 W_edge. b_edge + causal/key
mask live in one host tensor mcolh; bias+mask adds ride the PE as
identity-matmul deposits into the sim PSUM accumulation group. Softmax exp is
one batched ACT op per j-block over all 8 heads. The ss square runs
chunk-batched on ACT (Square) or GpSimd (tensor_tensor mult) per SS_MODE; the
256->1 reduce is one DVE tensor_reduce per chunk into contiguous fp16 (hits
the 2x_1P packed mode; all-2B dtypes required), scattered into fp32 ss_all by
GpSimd copies. q/k stay fp32 through sim for accuracy; v/attn are bf16.
den/av accumulation chains run in-stream on dedicated PSUM banks after each
sim block.
"""
import sys
sys.path.insert(0, "/opt/trn_rl_repo")
import numpy as np
import ml_dtypes

import concourse.bass as bass
import concourse.mybir as mybir
import concourse.tile as tile
from concourse.bass_utils import run_bass_kernel_spmd

B, N, DIM = 2, 512, 256
H, DH = 8, 64
EPS = 1e-5
NEG = -1e30
F32 = mybir.dt.float32
F16 = mybir.dt.float16
BF16 = mybir.dt.bfloat16
FP8 = mybir.dt.float8e4
BF = ml_dtypes.bfloat16
F8 = ml_dtypes.float8_e4m3fn

# Tiles enumerated ascending (r, t) -- matches the DRAM packing order.
# Chunks emitted descending (high rows first) so sim j-block t can start as
# soon as rows >= 32t are staged.
TILE_IDX = {}
TILE_REV = []
for _r in range(128):
    for _t in range(_r // 32 + 1):
        TILE_IDX[(_r, _t)] = len(TILE_REV)
        TILE_REV.append((_r, _t))
NT = len(TILE_REV)   # 320
CH = 32              # tiles per chunk
NCH = NT // CH       # 10 chunks

# ss/mcolh columns laid out t-grouped: block t occupies columns
# [TOFF[t], TOFF[t+1]) indexed by (r - 32t).
TOFF = [0, 128, 224, 288, 320]
NR = [128, 96, 64, 32]
def POS(r, t):
    return TOFF[t] + (r - 32 * t)

# per-chunk group structure: groups[t] = (t, sig0, q_list, r0, cnt)
CHUNK_GROUPS = []
for _c in range(NCH):
    by_t = {}
    for _q in range(CH):
        _r, _t = TILE_REV[CH * _c + _q]
        by_t.setdefault(_t, []).append((_q, _r))
    groups = []
    sig = 0
    for _t in sorted(by_t):
        qs = by_t[_t]
        r0 = qs[0][1]
        assert [r for _, r in qs] == list(range(r0, r0 + len(qs)))
        groups.append((_t, sig, [q for q, _ in qs], r0, len(qs)))
        sig += len(qs)
    CHUNK_GROUPS.append(groups)

# per-chunk square engine: 'a'=ACT batched Square, 'g'=GpSimd tensor_tensor
SS_MODE = "gagagagagg"
assert len(SS_MODE) == NCH

_ctr = [0]


def _nop_with_wait(engine, wait):
    _ctr[0] += 1
    n = mybir.InstNoOp.__new__(mybir.InstNoOp, name=f"waitnop-{_ctr[0]}")
    n.engine = engine
    n.sync_info = mybir.SyncInfo.__new__(mybir.SyncInfo, on_wait=[wait], on_update=[])
    return n


def split_waits(nc):
    """Walrus encodes at most ONE sem-wait per instruction; Tile attaches
    many. Move extras onto NOPs inserted just before, same engine."""
    for f in nc.m.functions:
        for b in f.blocks:
            out, changed = [], False
            for inst in b.instructions:
                si = inst.sync_info
                waits = list(si.on_wait) if (si and si.on_wait) else []
                keep = 0 if inst.opcode == "Drain" else 1
                if len(waits) > keep:
                    changed = True
                    moved = waits[:-keep] if keep else waits
                    kept = waits[-keep:] if keep else []
                    for w in moved:
                        out.append(_nop_with_wait(inst.engine, w))
                    inst.sync_info = mybir.SyncInfo.__new__(
                        mybir.SyncInfo, on_wait=kept,
                        on_update=list(si.on_update) if si.on_update else [])
                out.append(inst)
            if changed:
                b.instructions = out


def build(debug=False):
    nc = bass.Bass()
    ef8_ext = nc.declare_dram_parameter("ef8", [128, NCH, 2, CH, DIM], FP8,
                                        isOutput=False)
    x_ext = nc.declare_dram_parameter("xb", [N, DIM], F32, isOutput=False)
    xq_ext = nc.declare_dram_parameter("xq", [128, DIM], F32, isOutput=False)
    wq32_ext = nc.declare_dram_parameter("wq32", [128, 2, 8, 128], F32, isOutput=False)
    wv16_ext = nc.declare_dram_parameter("wv16", [128, 2, 4, 128], BF16, isOutput=False)
    we16_ext = nc.declare_dram_parameter("we16", [128, 2, H], BF16, isOutput=False)
    wo16_ext = nc.declare_dram_parameter("wo16", [128, 4, DIM], BF16, isOutput=False)
    mcolh_ext = nc.declare_dram_parameter("mcolh", [128, NT, H], BF16, isOutput=False)
    id_ext = nc.declare_dram_parameter("ident", [128, 128], BF16, isOutput=False)
    out_ext = nc.declare_dram_parameter("out", [128, DIM], F32, isOutput=True)
    if debug:
        dbg_ss = nc.declare_dram_parameter("dbg_ss", [128, NT], F32, isOutput=True)
        dbg_raw = [nc.declare_dram_parameter(f"dbg_raw{t}", [128, NR[t], H], F32,
                                             isOutput=True) for t in range(4)]
        dbg_den = nc.declare_dram_parameter("dbg_den", [1, 2, 512], F32, isOutput=True)
        dbg_av = nc.declare_dram_parameter("dbg_av", [128, H * DH], F32, isOutput=True)
        dbg_attn = nc.declare_dram_parameter("dbg_attn", [128, 4, H, 128], F32,
                                             isOutput=True)

    AF = mybir.ActivationFunctionType
    MUL, ADD = mybir.AluOpType.mult, mybir.AluOpType.add

    with tile.TileContext(nc) as tc:
        with tc.tile_pool(name="cst", bufs=1) as cst, \
             tc.tile_pool(name="ep", bufs=3) as ep, \
             tc.tile_pool(name="sqp", bufs=2) as sqp, \
             tc.tile_pool(name="ssp", bufs=2) as ssp, \
             tc.tile_pool(name="wk", bufs=2) as wk, \
             tc.tile_pool(name="bps", bufs=2, space="PSUM") as bps, \
             tc.tile_pool(name="sps", bufs=1, space="PSUM") as sps, \
             tc.tile_pool(name="dps", bufs=1, space="PSUM") as dps, \
             tc.tile_pool(name="mps", bufs=1, space="PSUM") as mps, \
             tc.tile_pool(name="avps", bufs=1, space="PSUM") as avps:

            # ---------------- constants ----------------
            ident = cst.tile([128, 128], BF16)
            nc.sync.dma_start(out=ident, in_=id_ext[:, :])
            we16 = cst.tile([128, 2, H], BF16)
            nc.sync.dma_start(out=we16, in_=we16_ext[:, :, :])
            wo16 = cst.tile([128, 4, DIM], BF16)
            nc.sync.dma_start(out=wo16, in_=wo16_ext[:, :, :])
            wq32 = cst.tile([128, 2, 8, 128], F32)
            nc.sync.dma_start(out=wq32, in_=wq32_ext[:, :, :, :])
            wv16 = cst.tile([128, 2, 4, 128], BF16)
            nc.sync.dma_start(out=wv16, in_=wv16_ext[:, :, :, :])
            mcolh = cst.tile([128, NT, H], BF16)
            nc.sync.dma_start(out=mcolh, in_=mcolh_ext[:, :, :])
            x32 = cst.tile([128, 5, DIM], F32)
            nc.sync.dma_start(out=x32[:, 0:4, :],
                              in_=x_ext.rearrange("(t p) d -> p t d", p=128))
            nc.sync.dma_start(out=x32[:, 4, :], in_=xq_ext[:, :])

            epsc = cst.tile([128, 1], F32)
            nc.vector.memset(epsc, EPS)
            ones16 = cst.tile([128, 1], BF16)
            nc.vector.memset(ones16, 1.0)
            onef = cst.tile([1, 1], F32)
            nc.vector.memset(onef, 1.0)

            ss_all = cst.tile([128, NT], F32)
            stgraw = []
            for t in range(4):
                sr = cst.tile([128, NR[t], H], BF16, tag=f"sraw{t}", name=f"sraw{t}")
                stgraw.append(sr)

            # ---------------- x path: rmsnorm, kT, qT, v ----------------
            ssx = wk.tile([128, 5], F32, tag="ssx")
            for t in range(5):
                dump = wk.tile([128, DIM], BF16, tag="sqdump")
                nc.scalar.activation(out=dump, in_=x32[:, t, :],
                                     func=AF.Square, accum_out=ssx[:, t:t + 1])
            sqm = wk.tile([128, 5], F32, tag="sqm")
            nc.scalar.activation(out=sqm, in_=ssx, func=AF.Ln,
                                 bias=epsc, scale=1.0 / DIM)
            rx = wk.tile([128, 5], F32, tag="rx")
            nc.scalar.activation(out=rx, in_=sqm, func=AF.Exp, scale=-0.5)
            xn32 = cst.tile([128, 5, DIM], F32)
            for t in range(5):
                nc.scalar.activation(out=xn32[:, t, :], in_=x32[:, t, :],
                                     func=AF.Copy, scale=rx[:, t:t + 1])
            ident32 = cst.tile([128, 128], F32)
            nc.vector.tensor_copy(ident32, ident)
            xnT32 = cst.tile([128, 2, 5, 128], F32)
            for t in range(5):
                ps32 = mps.tile([128, 2, 128], F32, tag="setup_ps")
                for kh in range(2):
                    nc.tensor.transpose(ps32[:, kh, :],
                                        xn32[:, t, kh * 128:(kh + 1) * 128], ident32)
                nc.scalar.copy(xnT32[:, :, t, :], ps32)
            xnT = cst.tile([128, 2, 5, 128], BF16)
            nc.gpsimd.tensor_copy(xnT, xnT32)

            kT = cst.tile([128, 4, N], F32)
            for ft in range(4):
                k_ps = mps.tile([128, N], F32, tag="setup_ps")
                for kh in range(2):
                    nc.tensor.matmul(k_ps,
                                     lhsT=wq32[:, kh, 4 + ft, :],
                                     rhs=xnT32[:, kh, 0:4, :].rearrange("p a b -> p (a b)"),
                                     start=(kh == 0), stop=(kh == 1))
                nc.scalar.copy(kT[:, ft, :], k_ps)
            qT = cst.tile([128, 4, 128], F32)
            for ft in range(4):
                q_ps = mps.tile([128, 128], F32, tag="setup_ps")
                for kh in range(2):
                    nc.tensor.matmul(q_ps, lhsT=wq32[:, kh, ft, :],
                                     rhs=xnT32[:, kh, 4, :],
                                     start=(kh == 0), stop=(kh == 1))
                nc.scalar.copy(qT[:, ft, :], q_ps)
            v16 = cst.tile([128, 4, H * DH], BF16)
            for st in range(4):
                v_ps = mps.tile([128, H * DH], F32, tag="setup_ps")
                for kh in range(2):
                    nc.tensor.matmul(v_ps,
                                     lhsT=xnT[:, kh, st, :],
                                     rhs=wv16[:, kh, :, :].rearrange("p a b -> p (a b)"),
                                     start=(kh == 0), stop=(kh == 1))
                nc.scalar.copy(v16[:, st, :], v_ps)

            av_ps = avps.tile([128, H * DH], F32, tag="avout")
            den_ps = []
            for g in range(2):
                dtile = dps.tile([1, 512], F32, tag=f"den{g}", name=f"den_ps{g}")
                den_ps.append(dtile)
            # attn tiles: [j, t, h, i]; masked region stays 0 from this memset
            attn_all = cst.tile([128, 4, H, 128], BF16)
            nc.gpsimd.memset(attn_all, 0.0)

            # ------------- edges + attention, chunks descending -------------

            def bcast8(ap2d, n):
                return ap2d.rearrange("p (c o) -> p c o", o=1).broadcast_to([128, n, H])

            def sim_block(t):
                n = NR[t]
                sl = slice(TOFF[t], TOFF[t + 1])
                # rinv = Exp(-0.5*Log(ms+eps)): Ln/Exp/Square share one ACT
                # table set
                srt = wk.tile([128, 128], F32, tag="srt")
                nc.scalar.activation(out=srt[:, 0:n], in_=ss_all[:, sl],
                                     func=AF.Ln, bias=epsc, scale=1.0 / DIM)
                rinv = wk.tile([128, 128], F32, tag="rinv")
                nc.scalar.activation(out=rinv[:, 0:n], in_=srt[:, 0:n],
                                     func=AF.Exp, scale=-0.5)
                stgtmp = wk.tile([128, 128, H], BF16, tag="stgtmp")
                nc.vector.tensor_mul(stgtmp[:, 0:n, :], stgraw[t],
                                     bcast8(rinv[:, 0:n], n))
                sim_all = sps.tile([128, H, 128], F32, tag="sim")
                for h in range(H):
                    pb, ft = (h % 2) * 64, h // 2
                    nc.tensor.matmul(sim_all[:, h, 0:n], lhsT=ident,
                                     rhs=stgtmp[:, 0:n, h],
                                     start=(h % 4 == 0), stop=False)
                    nc.tensor.matmul(sim_all[:, h, 0:n], lhsT=ident,
                                     rhs=mcolh[:, sl, h],
                                     start=False, stop=False)
                    nc.tensor.matmul(sim_all[:, h, 0:n],
                                     lhsT=kT[pb:pb + 64, ft, t * 128:(t + 1) * 128],
                                     rhs=qT[pb:pb + 64, ft, 32 * t:128],
                                     start=False, stop=True)
                nc.scalar.activation(out=attn_all[:, t, :, 32 * t:],
                                     in_=sim_all[:, :, 0:n], func=AF.Exp)
                for h in range(H):
                    nc.tensor.matmul(
                        den_ps[h // 4][0:1, (h % 4) * 128:(h % 4) * 128 + 128],
                        lhsT=ones16, rhs=attn_all[:, t, h, :],
                        start=(t == 3 and h % 4 == 0), stop=(t == 0))
                for h in range(H):
                    nc.tensor.matmul(av_ps[:, h * DH:(h + 1) * DH],
                                     lhsT=attn_all[:, t, h, :],
                                     rhs=v16[:, t, h * DH:(h + 1) * DH],
                                     start=(t == 3 and h == 0), stop=(t == 0))

            sig_of_q = []
            for c in range(NCH):
                m = [0] * CH
                for (t, sig0, qs, r0, cnt) in CHUNK_GROUPS[c]:
                    for i, q in enumerate(qs):
                        m[q] = sig0 + i
                sig_of_q.append(m)

            for c in range(NCH - 1, -1, -1):
                ec = ep.tile([128, 2, CH, DIM], FP8, tag="ec")
                nc.scalar.dma_start(out=ec, in_=ef8_ext[:, c, :, :, :])
                mode = SS_MODE[c]
                bias_chunk = bps.tile([128, CH, H], F32, tag="bias")
                sq16 = sqp.tile([128, CH, DIM], F16, tag="sq16")
                if mode == "a":
                    nc.scalar.activation(out=sq16, in_=ec[:, 0, :, :],
                                         func=AF.Square)
                else:
                    nc.gpsimd.tensor_tensor(sq16, ec[:, 0, :, :],
                                            ec[:, 0, :, :], op=MUL)
                ssc = ssp.tile([128, CH], F16, tag="ssc")
                with nc.allow_low_precision("fp16 ss staging; final ss is fp32"):
                    nc.vector.tensor_reduce(out=ssc, in_=sq16,
                                            axis=mybir.AxisListType.X, op=ADD)
                for (t, sig0, qs, r0, cnt) in CHUNK_GROUPS[c]:
                    p0 = POS(r0, t)
                    nc.gpsimd.tensor_copy(ss_all[:, p0:p0 + cnt],
                                          ssc[:, sig0:sig0 + cnt])
                for q in range(CH):
                    for kh in range(2):
                        nc.tensor.matmul(bias_chunk[:, sig_of_q[c][q], :],
                                         lhsT=ec[:, 1, q, kh * 128:(kh + 1) * 128],
                                         rhs=we16[:, kh, :],
                                         start=(kh == 0), stop=(kh == 1))
                for (t, sig0, qs, r0, cnt) in CHUNK_GROUPS[c]:
                    nc.vector.tensor_copy(
                        stgraw[t][:, r0 - 32 * t:r0 - 32 * t + cnt, :],
                        bias_chunk[:, sig0:sig0 + cnt, :])
                if c == 6:
                    sim_block(3)
                elif c == 3:
                    sim_block(2)
                elif c == 1:
                    sim_block(1)
            sim_block(0)

            # ---------------- epilogue ----------------
            den_sb = cst.tile([1, 2, 512], F32)
            for g in range(2):
                nc.scalar.copy(den_sb[:, g, :], den_ps[g])
            denT_ps = mps.tile([128, H], F32, tag="setup_ps")
            for h in range(H):
                nc.tensor.matmul(denT_ps[:, h:h + 1],
                                 lhsT=den_sb[0:1, h // 4, (h % 4) * 128:(h % 4) * 128 + 128],
                                 rhs=onef, start=True, stop=True)
            rv = cst.tile([128, H], F32)
            nc.vector.reciprocal(rv, denT_ps)
            av_sb = cst.tile([128, H * DH], BF16)
            for h in range(H):
                nc.vector.tensor_scalar(out=av_sb[:, h * DH:(h + 1) * DH],
                                        in0=av_ps[:, h * DH:(h + 1) * DH],
                                        scalar1=rv[:, h:h + 1], scalar2=None,
                                        op0=MUL)
            avT = cst.tile([128, 4, 128], BF16)
            nc.sync.dma_start(out=avT, in_=av_sb, transpose=True)
            out_ps = avps.tile([128, DIM], F32, tag="avout")
            for q4 in range(4):
                nc.tensor.matmul(out_ps, lhsT=avT[:, q4, :], rhs=wo16[:, q4, :],
                                 start=(q4 == 0), stop=(q4 == 3))
            out_sb = cst.tile([128, DIM], F32)
            nc.vector.tensor_copy(out_sb, out_ps)
            nc.sync.dma_start(out=out_ext[:, :], in_=out_sb)
            if debug:
                nc.sync.dma_start(out=dbg_ss[:, :], in_=ss_all)
                for t in range(4):
                    raw32 = cst.tile([128, NR[t], H], F32, tag=f"r32_{t}",
                                     name=f"r32_{t}")
                    nc.vector.tensor_copy(raw32, stgraw[t])
                    nc.sync.dma_start(out=dbg_raw[t][:, :, :], in_=raw32)
                nc.sync.dma_start(out=dbg_den[:, :, :], in_=den_sb)
                av32 = cst.tile([128, H * DH], F32)
                nc.vector.tensor_copy(av32, av_sb)
                nc.sync.dma_start(out=dbg_av[:, :], in_=av32)
                at32 = cst.tile([128, 4, H, 128], F32)
                nc.vector.tensor_copy(at32, attn_all)
                nc.sync.dma_start(out=dbg_attn[:, :, :, :], in_=at32)
    return nc


_NC_CACHE = [None]
LAST_RESULT = [None]


def _pack_core(edges_b8, x, b, l, mask, b_edge):
    """Per-core host packing: fp8 causal-prefix edges in two layouts + mcolh."""
    E8 = edges_b8[l::4]                       # [128, 512, 256] fp8
    e8_parts, eT_parts = [], []
    for r in range(128):
        jt = r // 32 + 1
        A = E8[r, :jt * 128, :]               # [jt*128, 256]
        e8_parts.append(A.reshape(jt, 128, DIM).transpose(1, 0, 2))
        AT = np.ascontiguousarray(A.T)        # [256, jt*128]
        eT_parts.append(AT.reshape(2, 128, jt, 128).transpose(1, 2, 0, 3))
    e8_all = np.concatenate(e8_parts, axis=1)     # [128, NT, 256]
    eT_all = np.concatenate(eT_parts, axis=1)     # [128, NT, 2, 128]
    ef8 = np.empty((128, NCH, 2, CH, DIM), dtype=F8)
    ef8[:, :, 0] = e8_all.reshape(128, NCH, CH, DIM)
    ef8[:, :, 1] = eT_all.reshape(128, NCH, CH, DIM)

    jj = np.arange(128)
    mcolh = np.empty((128, NT, H), np.float32)
    for r in range(128):
        for t in range(r // 32 + 1):
            j = t * 128 + jj
            valid = (j <= 4 * r + l) & mask[b, j]
            mcolh[:, POS(r, t), :] = np.where(valid[:, None], b_edge[None, :], NEG)
    xq = np.ascontiguousarray(x[b, l::4])
    return ef8, mcolh.astype(BF), xq


def kernel(x, mask, edges, gamma_x, W_qkv, gamma_e, W_edge, b_edge, W_out):
    x = np.asarray(x, np.float32)
    mask = np.asarray(mask)
    edges = np.asarray(edges, np.float32)
    gamma_x = np.asarray(gamma_x, np.float32)
    W_qkv = np.asarray(W_qkv, np.float32)
    gamma_e = np.asarray(gamma_e, np.float32)
    W_edge = np.asarray(W_edge, np.float32)
    b_edge = np.asarray(b_edge, np.float32)
    W_out = np.asarray(W_out, np.float32)

    wqkv_f = (gamma_x[:, None] * W_qkv).copy()
    wqkv_f[:, :H * DH] *= DH ** 0.5
    wq32 = np.ascontiguousarray(
        wqkv_f[:, :1024].reshape(2, 128, 8, 128).transpose(1, 0, 2, 3))
    wv16 = np.ascontiguousarray(
        wqkv_f[:, 1024:1536].reshape(2, 128, 4, 128).transpose(1, 0, 2, 3)
    ).astype(BF)
    wedge_f = gamma_e[:, None] * W_edge
    we16 = np.ascontiguousarray(
        wedge_f.reshape(2, 128, H).transpose(1, 0, 2)).astype(BF)
    wo16 = np.ascontiguousarray(
        W_out.reshape(4, 128, DIM).transpose(1, 0, 2)).astype(BF)
    ident = np.eye(128, dtype=BF)

    edges8 = np.clip(edges, -224.0, 224.0).astype(F8)   # [2, 512, 512, 256]

    in_maps = []
    for c in range(8):
        b, l = c // 4, c % 4
        ef8, mcolh, xq = _pack_core(edges8[b], x, b, l, mask, b_edge)
        in_maps.append({
            "ef8": ef8, "xb": x[b], "xq": xq,
            "wq32": wq32, "wv16": wv16, "we16": we16, "wo16": wo16,
            "mcolh": mcolh, "ident": ident,
        })

    if _NC_CACHE[0] is None:
        nc = build()
        split_waits(nc)
        _NC_CACHE[0] = nc
    res = run_bass_kernel_spmd(_NC_CACHE[0], in_maps, core_ids=list(range(8)))
    LAST_RESULT[0] = res

    out = np.zeros((B, N, DIM), np.float32)
    for c in range(8):
        b, l = c // 4, c % 4
        out[b, l::4] = res.results[c]["out"]
    return out


# revision 19
# speedup vs baseline: 1.2996x; 1.2996x over previous
"""Trainium2 Bass kernel v5 for nn_Attention_22050362097758 (edge-biased causal
attention; b=2, n=512, dim=256, heads=8, dim_head=64).

Sharding: core c -> batch c//4, lane l=c%4, query rows i = 4r+l (r=0..127).
Row-interleaving makes causal extents core-uniform (SPMD: one program).

v5: ONLY the d-on-partitions fp8 causal-prefix edge pack is loaded (10.5 MB
per core -- the j-layout ss copy is gone). The sum-of-squares rides the PE:
ACT/GpSimd/DVE square the d-layout slabs chunk-batched (per SS_MODE), then
per-slab ones-matmuls reduce over d with the squared slab as FWL weights,
depositing ss as a j-on-partitions PSUM column next to the bias columns
(bias_chunk[:, q, 8]). Tiles are t-major (chunk == one j-block slice) so sim
block t fires as soon as its 32-row chunk group lands; chunks ascend, sim
blocks at c==3/6/8/end. bias+mask adds run on DVE (stgfin then scr vs the sim
PSUM); exp is one batched ACT op per block over all 8 heads. den is fused
into the attn@v matmul via a ones-column appended to V (65-wide rhs), killing
the separate den chain + transpose. q/k stay fp32 through sim for accuracy.
"""
import sys, os
sys.path.insert(0, "/opt/trn_rl_repo")
KV = os.environ.get("KV", "full")
import numpy as np
import ml_dtypes

import concourse.bass as bass
import concourse.mybir as mybir
import concourse.tile as tile
from concourse.bass_utils import run_bass_kernel_spmd

B, N, DIM = 2, 512, 256
H, DH = 8, 64
EPS = 1e-5
NEG = -1e30
F32 = mybir.dt.float32
F16 = mybir.dt.float16
BF16 = mybir.dt.bfloat16
FP8 = mybir.dt.float8e4
BF = ml_dtypes.bfloat16
F8 = ml_dtypes.float8_e4m3fn

# t-major tile order: global tile index k = POS(r, t) = TOFF[t] + (r - 32*t).
# Block t occupies [TOFF[t], TOFF[t+1]); chunks of 32 tiles align exactly.
TOFF = [0, 128, 224, 288, 320]
NR = [128, 96, 64, 32]
NT = 320
CH = 32
NCH = NT // CH   # 10
CHUNK_T = [0, 0, 0, 0, 1, 1, 1, 2, 2, 3]


def POS(r, t):
    return TOFF[t] + (r - 32 * t)


# per-chunk square engine: 'a'=ACT Square (fp8 out), 'g'=GpSimd tensor_tensor
# (bf16 out), 'v'=DVE tensor_tensor (bf16 out)
SS_MODE = "agvagagvag"
assert len(SS_MODE) == NCH

_ctr = [0]


def _nop_with_wait(engine, wait):
    _ctr[0] += 1
    n = mybir.InstNoOp.__new__(mybir.InstNoOp, name=f"waitnop-{_ctr[0]}")
    n.engine = engine
    n.sync_info = mybir.SyncInfo.__new__(mybir.SyncInfo, on_wait=[wait], on_update=[])
    return n


def split_waits(nc):
    """Walrus encodes at most ONE sem-wait per instruction; Tile attaches
    many. Move extras onto NOPs inserted just before, same engine."""
    for f in nc.m.functions:
        for b in f.blocks:
            out, changed = [], False
            for inst in b.instructions:
                si = inst.sync_info
                waits = list(si.on_wait) if (si and si.on_wait) else []
                keep = 0 if inst.opcode == "Drain" else 1
                if len(waits) > keep:
                    changed = True
                    moved = waits[:-keep] if keep else waits
                    kept = waits[-keep:] if keep else []
                    for w in moved:
                        out.append(_nop_with_wait(inst.engine, w))
                    inst.sync_info = mybir.SyncInfo.__new__(
                        mybir.SyncInfo, on_wait=kept,
                        on_update=list(si.on_update) if si.on_update else [])
                out.append(inst)
            if changed:
                b.instructions = out


def build(debug=False):
    nc = bass.Bass()
    ef8_ext = nc.declare_dram_parameter("ef8", [128, NCH, CH, DIM], FP8,
                                        isOutput=False)
    x_ext = nc.declare_dram_parameter("xb", [N, DIM], F32, isOutput=False)
    xq_ext = nc.declare_dram_parameter("xq", [128, DIM], F32, isOutput=False)
    wq32_ext = nc.declare_dram_parameter("wq32", [128, 2, 8, 128], F32, isOutput=False)
    wv16_ext = nc.declare_dram_parameter("wv16", [128, 2, 4, 128], BF16, isOutput=False)
    we16_ext = nc.declare_dram_parameter("we16", [128, 2, H], BF16, isOutput=False)
    wo16_ext = nc.declare_dram_parameter("wo16", [128, 4, DIM], BF16, isOutput=False)
    mcolh_ext = nc.declare_dram_parameter("mcolh", [128, NT, H], BF16, isOutput=False)
    id_ext = nc.declare_dram_parameter("ident", [128, 128], BF16, isOutput=False)
    out_ext = nc.declare_dram_parameter("out", [128, DIM], F32, isOutput=True)
    if debug:
        dbg_ss = nc.declare_dram_parameter("dbg_ss", [128, NT], F32, isOutput=True)
        dbg_raw = [nc.declare_dram_parameter(f"dbg_raw{t}", [128, NR[t], H], F32,
                                             isOutput=True) for t in range(4)]
        dbg_av = nc.declare_dram_parameter("dbg_av", [128, H * DH], F32, isOutput=True)
        dbg_attn = nc.declare_dram_parameter("dbg_attn", [128, 4, H, 128], F32,
                                             isOutput=True)

    AF = mybir.ActivationFunctionType
    MUL, ADD = mybir.AluOpType.mult, mybir.AluOpType.add

    with tile.TileContext(nc) as tc:
        with tc.tile_pool(name="cst", bufs=1) as cst, \
             tc.tile_pool(name="ep", bufs=2) as ep, \
             tc.tile_pool(name="sqp", bufs=2) as sqp, \
             tc.tile_pool(name="wk", bufs=2) as wk, \
             tc.tile_pool(name="bps", bufs=2, space="PSUM") as bps, \
             tc.tile_pool(name="sps", bufs=1, space="PSUM") as sps, \
             tc.tile_pool(name="mps", bufs=1, space="PSUM") as mps, \
             tc.tile_pool(name="avps", bufs=1, space="PSUM") as avps:

            # ---------------- constants ----------------
            ident = cst.tile([128, 128], BF16)
            nc.sync.dma_start(out=ident, in_=id_ext[:, :])
            we16 = cst.tile([128, 2, H], BF16)
            nc.sync.dma_start(out=we16, in_=we16_ext[:, :, :])
            wo16 = cst.tile([128, 4, DIM], BF16)
            nc.sync.dma_start(out=wo16, in_=wo16_ext[:, :, :])
            wq32 = cst.tile([128, 2, 8, 128], F32)
            nc.sync.dma_start(out=wq32, in_=wq32_ext[:, :, :, :])
            wv16 = cst.tile([128, 2, 4, 128], BF16)
            nc.sync.dma_start(out=wv16, in_=wv16_ext[:, :, :, :])
            mcolh = cst.tile([128, NT, H], BF16)
            nc.sync.dma_start(out=mcolh, in_=mcolh_ext[:, :, :])
            x32 = cst.tile([128, 5, DIM], F32)
            nc.sync.dma_start(out=x32[:, 0:4, :],
                              in_=x_ext.rearrange("(t p) d -> p t d", p=128))
            nc.sync.dma_start(out=x32[:, 4, :], in_=xq_ext[:, :])

            epsc = cst.tile([128, 1], F32)
            nc.vector.memset(epsc, EPS)
            ones16 = cst.tile([128, 1], BF16)
            nc.vector.memset(ones16, 1.0)

            ss_all = cst.tile([128, NT], F32)
            if KV == "noss":
                nc.vector.memset(ss_all, 256.0)
            stgraw = []
            for t in range(4):
                sr = cst.tile([128, NR[t], H], BF16, tag=f"sraw{t}", name=f"sraw{t}")
                stgraw.append(sr)

            # ---------------- x path: rmsnorm, kT, qT, v ----------------
            ssx = wk.tile([128, 5], F32, tag="ssx")
            for t in range(5):
                dump = wk.tile([128, DIM], BF16, tag="sqdump")
                nc.scalar.activation(out=dump, in_=x32[:, t, :],
                                     func=AF.Square, accum_out=ssx[:, t:t + 1])
            sqm = wk.tile([128, 5], F32, tag="sqm")
            nc.scalar.activation(out=sqm, in_=ssx, func=AF.Ln,
                                 bias=epsc, scale=1.0 / DIM)
            rx = wk.tile([128, 5], F32, tag="rx")
            nc.scalar.activation(out=rx, in_=sqm, func=AF.Exp, scale=-0.5)
            xn32 = cst.tile([128, 5, DIM], F32)
            for t in range(5):
                nc.scalar.activation(out=xn32[:, t, :], in_=x32[:, t, :],
                                     func=AF.Copy, scale=rx[:, t:t + 1])
            ident32 = cst.tile([128, 128], F32)
            nc.vector.tensor_copy(ident32, ident)
            xnT32 = cst.tile([128, 2, 5, 128], F32)
            for t in range(5):
                ps32 = mps.tile([128, 2, 128], F32, tag="setup_ps")
                for kh in range(2):
                    nc.tensor.transpose(ps32[:, kh, :],
                                        xn32[:, t, kh * 128:(kh + 1) * 128], ident32)
                nc.scalar.copy(xnT32[:, :, t, :], ps32)
            xnT = cst.tile([128, 2, 5, 128], BF16)
            nc.vector.tensor_copy(xnT, xnT32)

            kT = cst.tile([128, 4, N], F32)
            for ft in range(4):
                k_ps = mps.tile([128, N], F32, tag="setup_ps")
                for kh in range(2):
                    nc.tensor.matmul(k_ps,
                                     lhsT=wq32[:, kh, 4 + ft, :],
                                     rhs=xnT32[:, kh, 0:4, :].rearrange("p a b -> p (a b)"),
                                     start=(kh == 0), stop=(kh == 1))
                nc.scalar.copy(kT[:, ft, :], k_ps)
            qT = cst.tile([128, 4, 128], F32)
            for ft in range(4):
                q_ps = mps.tile([128, 128], F32, tag="setup_ps")
                for kh in range(2):
                    nc.tensor.matmul(q_ps, lhsT=wq32[:, kh, ft, :],
                                     rhs=xnT32[:, kh, 4, :],
                                     start=(kh == 0), stop=(kh == 1))
                nc.scalar.copy(qT[:, ft, :], q_ps)
            # V with a ones-column appended per head: attn @ [v | 1] gives
            # av in cols 0:64 and the softmax denominator in col 64.
            v16e = cst.tile([128, 4, H, DH + 1], BF16)
            nc.vector.memset(v16e.rearrange("p a h d -> p (a h d)"), 1.0)
            NV = DH if KV == "nov16e" else DH + 1
            for st in range(4):
                v_ps = mps.tile([128, H * DH], F32, tag="setup_ps")
                for kh in range(2):
                    nc.tensor.matmul(v_ps,
                                     lhsT=xnT[:, kh, st, :],
                                     rhs=wv16[:, kh, :, :].rearrange("p a b -> p (a b)"),
                                     start=(kh == 0), stop=(kh == 1))
                if KV == "nov16e":
                    nc.scalar.copy(
                        v16e[:, st, :, 0:DH].rearrange("p h d -> p h d"),
                        v_ps.rearrange("p (h d) -> p h d", h=H))
                else:
                    nc.scalar.copy(v16e[:, st, :, 0:DH],
                                   v_ps.rearrange("p (h d) -> p h d", h=H))

            # av accumulates across blocks in SBUF (PSUM allows only one
            # pending accumulation group per bank, so long-lived per-head
            # PSUM chains are structurally unsound).
            avacc = cst.tile([128, H, DH + 2], F32)
            nc.vector.memset(avacc.rearrange("p h d -> p (h d)"), 0.0)
            # attn tiles: [j, t, h, i]; masked region stays 0 from this memset
            attn_all = cst.tile([128, 4, H, 128], BF16)
            nc.vector.memset(attn_all.rearrange("p a h d -> p (a h d)"), 0.0)

            # ------------- edges + attention, chunks ascending -------------

            def bcast8(ap2d, n):
                return ap2d.rearrange("p (c o) -> p c o", o=1).broadcast_to([128, n, H])

            def sim_block(t):
                n = NR[t]
                sl = slice(TOFF[t], TOFF[t + 1])
                # rinv = Exp(-0.5*Log(ms+eps)): Ln/Exp/Square share one ACT
                # table set
                srt = wk.tile([128, 128], F32, tag="srt")
                nc.scalar.activation(out=srt[:, 0:n], in_=ss_all[:, sl],
                                     func=AF.Ln, bias=epsc, scale=1.0 / DIM)
                rinv = wk.tile([128, 128], F32, tag="rinv")
                nc.scalar.activation(out=rinv[:, 0:n], in_=srt[:, 0:n],
                                     func=AF.Exp, scale=-0.5)
                stgtmp = wk.tile([128, 128, H], BF16, tag="stgtmp")
                nc.vector.tensor_mul(stgtmp[:, 0:n, :], stgraw[t],
                                     bcast8(rinv[:, 0:n], n))
                # stgfin stored h-major so the scr add is ONE op over the
                # whole sim tile -- a per-h DVE read would race the PE still
                # writing later heads in the same PSUM bank (fatal).
                stgfin = wk.tile([128, H, 128], BF16, tag="stgfin")
                if KV == "nostg":
                    for h in range(H):
                        nc.vector.tensor_add(stgfin[:, h, 0:n],
                                             stgtmp[:, 0:n, h],
                                             mcolh[:, sl, h])
                else:
                    nc.vector.tensor_add(
                        stgfin[:, :, 0:n].rearrange("p h n -> p n h"),
                        stgtmp[:, 0:n, :], mcolh[:, sl, :])
                if KV == "justtmp":
                    return
                sim_ps = sps.tile([128, H, 128], F32, tag="sim")
                for h in range(H):
                    pb, ft = (h % 2) * 64, h // 2
                    nc.tensor.matmul(sim_ps[:, h, 0:n],
                                     lhsT=kT[pb:pb + 64, ft, t * 128:(t + 1) * 128],
                                     rhs=qT[pb:pb + 64, ft, 32 * t:128],
                                     start=True, stop=True)
                scr = wk.tile([128, H, 128], F32, tag="scr")
                nc.vector.tensor_add(scr[:, :, 0:n], sim_ps[:, :, 0:n],
                                     stgfin[:, :, 0:n])
                nc.scalar.activation(out=attn_all[:, t, :, 32 * t:],
                                     in_=scr[:, :, 0:n], func=AF.Exp)
                if KV == "noav":
                    return
                avb = []
                for g in range(2):
                    avb.append(avps.tile([128, 4, DH + 2], F32, tag=f"av{g}",
                                         name=f"avb{g}_{t}"))
                for h in range(H):
                    nc.tensor.matmul(avb[h // 4][:, h % 4, 0:NV],
                                     lhsT=attn_all[:, t, h, :],
                                     rhs=v16e[:, t, h, 0:NV],
                                     start=True, stop=True)
                for g in range(2):
                    nc.vector.tensor_add(avacc[:, 4 * g:4 * g + 4, 0:DH + 1],
                                         avacc[:, 4 * g:4 * g + 4, 0:DH + 1],
                                         avb[g][:, :, 0:DH + 1])

            for c in range(NCH):
                tb = CHUNK_T[c]
                lo = 32 * c - TOFF[tb]          # local row offset in block tb
                ec = ep.tile([128, CH, DIM], FP8, tag="ec")
                nc.scalar.dma_start(out=ec, in_=ef8_ext[:, c, :, :])
                mode = SS_MODE[c]
                if mode == "a":
                    sq = sqp.tile([128, CH, DIM], BF16, tag="sqg")
                    nc.scalar.activation(out=sq, in_=ec, func=AF.Square)
                elif mode == "g":
                    sq = sqp.tile([128, CH, DIM], BF16, tag="sqg")
                    nc.gpsimd.tensor_tensor(sq, ec, ec, op=MUL)
                else:
                    sq = sqp.tile([128, CH, DIM], BF16, tag="sqg")
                    nc.vector.tensor_tensor(sq, ec, ec, op=MUL)
                bias_chunk = bps.tile([128, CH, 10], F32, tag="bias")
                for q in range(CH):
                    if KV != "noss":
                        for kh in range(2):
                            nc.tensor.matmul(bias_chunk[:, q, 8:9],
                                             lhsT=sq[:, q, kh * 128:(kh + 1) * 128],
                                             rhs=ones16,
                                             start=(kh == 0), stop=(kh == 1))
                    for kh in range(2):
                        nc.tensor.matmul(bias_chunk[:, q, 0:8],
                                         lhsT=ec[:, q, kh * 128:(kh + 1) * 128],
                                         rhs=we16[:, kh, :],
                                         start=(kh == 0), stop=(kh == 1))
                nc.vector.tensor_copy(stgraw[tb][:, lo:lo + CH, :],
                                      bias_chunk[:, :, 0:8])
                if KV != "noss":
                    nc.vector.tensor_copy(ss_all[:, 32 * c:32 * (c + 1)],
                                          bias_chunk[:, :, 8])
                if KV != "chunksonly":
                    if c == 3:
                        sim_block(0)
                    elif c == 6:
                        sim_block(1)
                    elif c == 8:
                        sim_block(2)
            if KV != "chunksonly":
                sim_block(3)

            # ---------------- epilogue ----------------
            rv = cst.tile([128, H], F32)
            if KV == "justtmp":
                out_sb0 = cst.tile([128, DIM], F32)
                nc.vector.tensor_copy(out_sb0, ss_all[:, 0:DIM])
                nc.sync.dma_start(out=out_ext[:, :], in_=out_sb0)
            if KV == "noav":
                out_sb0 = cst.tile([128, DIM], F32)
                nc.vector.tensor_copy(
                    out_sb0, attn_all[:, 0, 0:2, :].rearrange("p a b -> p (a b)"))
                nc.sync.dma_start(out=out_ext[:, :], in_=out_sb0)
            if KV == "chunksonly":
                out_sb0 = cst.tile([128, DIM], F32)
                nc.vector.tensor_copy(out_sb0, stgraw[0][:, 0:32, :].rearrange("p a b -> p (a b)"))
                nc.sync.dma_start(out=out_ext[:, :], in_=out_sb0)
            if KV == "nov16e":
                nc.vector.memset(rv, 1.0)
            elif KV not in ("chunksonly", "noav", "justtmp"):
                nc.vector.reciprocal(rv, avacc[:, :, DH])
            av_sb = cst.tile([128, H * DH], BF16)
            for h in (range(H) if KV not in ("chunksonly", "noav", "justtmp") else []):
                nc.vector.tensor_scalar(out=av_sb[:, h * DH:(h + 1) * DH],
                                        in0=avacc[:, h, 0:DH],
                                        scalar1=rv[:, h:h + 1], scalar2=None,
                                        op0=MUL)
            if KV not in ("chunksonly", "noav", "justtmp"):
                avT = cst.tile([128, 4, 128], BF16)
                nc.sync.dma_start(out=avT, in_=av_sb, transpose=True)
                out_ps = mps.tile([128, DIM], F32, tag="outp")
                for q4 in range(4):
                    nc.tensor.matmul(out_ps, lhsT=avT[:, q4, :],
                                     rhs=wo16[:, q4, :],
                                     start=(q4 == 0), stop=(q4 == 3))
                out_sb = cst.tile([128, DIM], F32)
                nc.vector.tensor_copy(out_sb, out_ps)
                nc.sync.dma_start(out=out_ext[:, :], in_=out_sb)
            if debug:
                nc.sync.dma_start(out=dbg_ss[:, :], in_=ss_all)
                for t in range(4):
                    raw32 = cst.tile([128, NR[t], H], F32, tag=f"r32_{t}",
                                     name=f"r32_{t}")
                    nc.vector.tensor_copy(raw32, stgraw[t])
                    nc.sync.dma_start(out=dbg_raw[t][:, :, :], in_=raw32)
                av32 = cst.tile([128, H * DH], F32)
                nc.vector.tensor_copy(av32, av_sb)
                nc.sync.dma_start(out=dbg_av[:, :], in_=av32)
                at32 = cst.tile([128, 4, H, 128], F32)
                nc.vector.tensor_copy(at32, attn_all)
                nc.sync.dma_start(out=dbg_attn[:, :, :, :], in_=at32)
    return nc


_NC_CACHE = [None]
LAST_RESULT = [None]


def _pack_core(edges_b8, x, b, l, mask, b_edge):
    """Per-core host packing: fp8 causal-prefix edges, d-on-partitions,
    t-major tile order; plus the mask/b_edge tensor mcolh."""
    E8 = edges_b8[l::4]                       # [128, 512, 256] fp8
    eT_all = np.empty((128, NT, DIM), dtype=F8)
    mcolh = np.empty((128, NT, H), np.float32)
    jj = np.arange(128)
    for t in range(4):
        nr = NR[t]
        blk = E8[32 * t:, 128 * t:128 * (t + 1), :]        # [nr, 128, 256]
        # [p, r, kh, j] <- blk[r, j, kh*128+p]
        eT_all[:, TOFF[t]:TOFF[t] + nr, :] = (
            blk.transpose(2, 0, 1).reshape(2, 128, nr, 128)
            .transpose(1, 2, 0, 3).reshape(128, nr, DIM))
        r = np.arange(32 * t, 128)
        valid = (128 * t + jj[:, None] <= 4 * r[None, :] + l) \
            & mask[b, 128 * t + jj][:, None]               # [128, nr]
        mcolh[:, TOFF[t]:TOFF[t] + nr, :] = np.where(
            valid[:, :, None], b_edge[None, None, :], NEG)
    ef8 = np.ascontiguousarray(eT_all.reshape(128, NCH, CH, DIM))
    xq = np.ascontiguousarray(x[b, l::4])
    return ef8, mcolh.astype(BF), xq


def kernel(x, mask, edges, gamma_x, W_qkv, gamma_e, W_edge, b_edge, W_out):
    x = np.asarray(x, np.float32)
    mask = np.asarray(mask)
    edges = np.asarray(edges, np.float32)
    gamma_x = np.asarray(gamma_x, np.float32)
    W_qkv = np.asarray(W_qkv, np.float32)
    gamma_e = np.asarray(gamma_e, np.float32)
    W_edge = np.asarray(W_edge, np.float32)
    b_edge = np.asarray(b_edge, np.float32)
    W_out = np.asarray(W_out, np.float32)

    wqkv_f = (gamma_x[:, None] * W_qkv).copy()
    wqkv_f[:, :H * DH] *= DH ** 0.5
    wq32 = np.ascontiguousarray(
        wqkv_f[:, :1024].reshape(2, 128, 8, 128).transpose(1, 0, 2, 3))
    wv16 = np.ascontiguousarray(
        wqkv_f[:, 1024:1536].reshape(2, 128, 4, 128).transpose(1, 0, 2, 3)
    ).astype(BF)
    wedge_f = gamma_e[:, None] * W_edge
    we16 = np.ascontiguousarray(
        wedge_f.reshape(2, 128, H).transpose(1, 0, 2)).astype(BF)
    wo16 = np.ascontiguousarray(
        W_out.reshape(4, 128, DIM).transpose(1, 0, 2)).astype(BF)
    ident = np.eye(128, dtype=BF)

    # |e| <= 15 so e^2 <= 225 stays under TRN fp8e4's 240 max-normal
    edges8 = np.clip(edges, -15.0, 15.0).astype(F8)

    in_maps = []
    for c in range(8):
        b, l = c // 4, c % 4
        ef8, mcolh, xq = _pack_core(edges8[b], x, b, l, mask, b_edge)
        in_maps.append({
            "ef8": ef8, "xb": x[b], "xq": xq,
            "wq32": wq32, "wv16": wv16, "we16": we16, "wo16": wo16,
            "mcolh": mcolh, "ident": ident,
        })

    if _NC_CACHE[0] is None:
        nc = build()
        split_waits(nc)
        _NC_CACHE[0] = nc
    res = run_bass_kernel_spmd(_NC_CACHE[0], in_maps, core_ids=list(range(8)))
    LAST_RESULT[0] = res

    out = np.zeros((B, N, DIM), np.float32)
    for c in range(8):
        b, l = c // 4, c % 4
        out[b, l::4] = res.results[c]["out"]
    return out


# revision 23
# speedup vs baseline: 1.3530x; 1.0411x over previous
"""Trainium2 Bass kernel v5 for nn_Attention_22050362097758 (edge-biased causal
attention; b=2, n=512, dim=256, heads=8, dim_head=64).

Sharding: core c -> batch c//4, lane l=c%4, query rows i = 4r+l (r=0..127).
Row-interleaving makes causal extents core-uniform (SPMD: one program).

v5: ONLY the d-on-partitions fp8 causal-prefix edge pack is loaded (10.5 MB
per core -- the j-layout ss copy is gone). The sum-of-squares rides the PE:
ACT/GpSimd/DVE square the d-layout slabs chunk-batched (per SS_MODE), then
per-slab ones-matmuls reduce over d with the squared slab as FWL weights,
depositing ss as a j-on-partitions PSUM column next to the bias columns
(bias_chunk[:, q, 8]). Tiles are t-major (chunk == one j-block slice) so sim
block t fires as soon as its 32-row chunk group lands; chunks ascend, sim
blocks at c==3/6/8/end. bias+mask adds run on DVE (stgfin then scr vs the sim
PSUM); exp is one batched ACT op per block over all 8 heads. den is fused
into the attn@v matmul via a ones-column appended to V (65-wide rhs), killing
the separate den chain + transpose. q/k stay fp32 through sim for accuracy.
"""
import sys, os
sys.path.insert(0, "/opt/trn_rl_repo")
KV = os.environ.get("KV", "full")
KSIM = int(os.environ.get("KSIM", "8"))
import numpy as np
import ml_dtypes

import concourse.bass as bass
import concourse.mybir as mybir
import concourse.tile as tile
from concourse.bass_utils import run_bass_kernel_spmd

B, N, DIM = 2, 512, 256
H, DH = 8, 64
EPS = 1e-5
NEG = -1e30
F32 = mybir.dt.float32
F16 = mybir.dt.float16
BF16 = mybir.dt.bfloat16
FP8 = mybir.dt.float8e4
BF = ml_dtypes.bfloat16
F8 = ml_dtypes.float8_e4m3fn

# t-major tile order: global tile index k = POS(r, t) = TOFF[t] + (r - 32*t).
# Block t occupies [TOFF[t], TOFF[t+1]); chunks of 32 tiles align exactly.
TOFF = [0, 128, 224, 288, 320]
NR = [128, 96, 64, 32]
NT = 320
CH = 32
NCH = NT // CH   # 10
CHUNK_T = [0, 0, 0, 0, 1, 1, 1, 2, 2, 3]


def POS(r, t):
    return TOFF[t] + (r - 32 * t)


# per-chunk square engine: 'a'=ACT Square (fp8 out), 'g'=GpSimd tensor_tensor
# (bf16 out), 'v'=DVE tensor_tensor (bf16 out)
SS_MODE = "agvagagvag"
assert len(SS_MODE) == NCH

_ctr = [0]


def _nop_with_wait(engine, wait):
    _ctr[0] += 1
    n = mybir.InstNoOp.__new__(mybir.InstNoOp, name=f"waitnop-{_ctr[0]}")
    n.engine = engine
    n.sync_info = mybir.SyncInfo.__new__(mybir.SyncInfo, on_wait=[wait], on_update=[])
    return n


def split_waits(nc):
    """Walrus encodes at most ONE sem-wait per instruction; Tile attaches
    many. Move extras onto NOPs inserted just before, same engine."""
    for f in nc.m.functions:
        for b in f.blocks:
            out, changed = [], False
            for inst in b.instructions:
                si = inst.sync_info
                waits = list(si.on_wait) if (si and si.on_wait) else []
                keep = 0 if inst.opcode == "Drain" else 1
                if len(waits) > keep:
                    changed = True
                    moved = waits[:-keep] if keep else waits
                    kept = waits[-keep:] if keep else []
                    for w in moved:
                        out.append(_nop_with_wait(inst.engine, w))
                    inst.sync_info = mybir.SyncInfo.__new__(
                        mybir.SyncInfo, on_wait=kept,
                        on_update=list(si.on_update) if si.on_update else [])
                out.append(inst)
            if changed:
                b.instructions = out


def build(debug=False):
    nc = bass.Bass()
    ef8_ext = nc.declare_dram_parameter("ef8", [128, NCH, CH, DIM], FP8,
                                        isOutput=False)
    x_ext = nc.declare_dram_parameter("xb", [N, DIM], F32, isOutput=False)
    xq_ext = nc.declare_dram_parameter("xq", [128, DIM], F32, isOutput=False)
    wq32_ext = nc.declare_dram_parameter("wq32", [128, 2, 8, 128], F32, isOutput=False)
    wv16_ext = nc.declare_dram_parameter("wv16", [128, 2, 4, 128], BF16, isOutput=False)
    we16_ext = nc.declare_dram_parameter("we16", [128, 2, H], BF16, isOutput=False)
    wo16_ext = nc.declare_dram_parameter("wo16", [128, 4, DIM], BF16, isOutput=False)
    mcolh_ext = nc.declare_dram_parameter("mcolh", [128, NT, H], BF16, isOutput=False)
    id_ext = nc.declare_dram_parameter("ident", [128, 128], BF16, isOutput=False)
    out_ext = nc.declare_dram_parameter("out", [128, DIM], F32, isOutput=True)
    if debug:
        dbg_ss = nc.declare_dram_parameter("dbg_ss", [128, NT], F32, isOutput=True)
        dbg_raw = [nc.declare_dram_parameter(f"dbg_raw{t}", [128, NR[t], H], F32,
                                             isOutput=True) for t in range(4)]
        dbg_av = nc.declare_dram_parameter("dbg_av", [128, H * DH], F32, isOutput=True)
        dbg_attn = nc.declare_dram_parameter("dbg_attn", [128, 4, H, 128], F32,
                                             isOutput=True)

    AF = mybir.ActivationFunctionType
    MUL, ADD = mybir.AluOpType.mult, mybir.AluOpType.add

    with tile.TileContext(nc) as tc:
        with tc.tile_pool(name="cst", bufs=1) as cst, \
             tc.tile_pool(name="ep", bufs=2) as ep, \
             tc.tile_pool(name="sqp", bufs=2) as sqp, \
             tc.tile_pool(name="wk", bufs=2) as wk, \
             tc.tile_pool(name="bps", bufs=2, space="PSUM") as bps, \
             tc.tile_pool(name="sps", bufs=1, space="PSUM") as sps, \
             tc.tile_pool(name="mps", bufs=1, space="PSUM") as mps, \
             tc.tile_pool(name="avps", bufs=1, space="PSUM") as avps:

            # ---------------- constants ----------------
            ident = cst.tile([128, 128], BF16)
            nc.sync.dma_start(out=ident, in_=id_ext[:, :])
            we16 = cst.tile([128, 2, H], BF16)
            nc.sync.dma_start(out=we16, in_=we16_ext[:, :, :])
            wo16 = cst.tile([128, 4, DIM], BF16)
            nc.sync.dma_start(out=wo16, in_=wo16_ext[:, :, :])
            wq32 = cst.tile([128, 2, 8, 128], F32)
            nc.sync.dma_start(out=wq32, in_=wq32_ext[:, :, :, :])
            wv16 = cst.tile([128, 2, 4, 128], BF16)
            nc.sync.dma_start(out=wv16, in_=wv16_ext[:, :, :, :])
            mcolh = cst.tile([128, NT, H], BF16)
            nc.sync.dma_start(out=mcolh, in_=mcolh_ext[:, :, :])
            x32 = cst.tile([128, 5, DIM], F32)
            nc.sync.dma_start(out=x32[:, 0:4, :],
                              in_=x_ext.rearrange("(t p) d -> p t d", p=128))
            nc.sync.dma_start(out=x32[:, 4, :], in_=xq_ext[:, :])

            epsc = cst.tile([128, 1], F32)
            nc.vector.memset(epsc, EPS)
            ones16 = cst.tile([128, 1], BF16)
            nc.vector.memset(ones16, 1.0)

            ss_all = cst.tile([128, NT], F32)
            if KV == "noss":
                nc.vector.memset(ss_all, 256.0)
            stgraw = []
            for t in range(4):
                sr = cst.tile([128, NR[t], H], BF16, tag=f"sraw{t}", name=f"sraw{t}")
                stgraw.append(sr)

            # ---------------- x path: rmsnorm, kT, qT, v ----------------
            ssx = wk.tile([128, 5], F32, tag="ssx")
            for t in range(5):
                dump = wk.tile([128, DIM], BF16, tag="sqdump")
                nc.scalar.activation(out=dump, in_=x32[:, t, :],
                                     func=AF.Square, accum_out=ssx[:, t:t + 1])
            sqm = wk.tile([128, 5], F32, tag="sqm")
            nc.scalar.activation(out=sqm, in_=ssx, func=AF.Ln,
                                 bias=epsc, scale=1.0 / DIM)
            rx = wk.tile([128, 5], F32, tag="rx")
            nc.scalar.activation(out=rx, in_=sqm, func=AF.Exp, scale=-0.5)
            xn32 = cst.tile([128, 5, DIM], F32)
            for t in range(5):
                nc.scalar.activation(out=xn32[:, t, :], in_=x32[:, t, :],
                                     func=AF.Copy, scale=rx[:, t:t + 1])
            ident32 = cst.tile([128, 128], F32)
            nc.vector.tensor_copy(ident32, ident)
            xnT32 = cst.tile([128, 2, 5, 128], F32)
            for t in range(5):
                ps32 = mps.tile([128, 2, 128], F32, tag="setup_ps")
                for kh in range(2):
                    nc.tensor.transpose(ps32[:, kh, :],
                                        xn32[:, t, kh * 128:(kh + 1) * 128], ident32)
                nc.scalar.copy(xnT32[:, :, t, :], ps32)
            xnT = cst.tile([128, 2, 5, 128], BF16)
            nc.vector.tensor_copy(xnT, xnT32)

            kT = cst.tile([128, 4, N], BF16)
            for ft in range(4):
                k_ps = mps.tile([128, N], F32, tag="setup_ps")
                for kh in range(2):
                    nc.tensor.matmul(k_ps,
                                     lhsT=wq32[:, kh, 4 + ft, :],
                                     rhs=xnT32[:, kh, 0:4, :].rearrange("p a b -> p (a b)"),
                                     start=(kh == 0), stop=(kh == 1))
                nc.scalar.copy(kT[:, ft, :], k_ps)
            qT = cst.tile([128, 4, 128], BF16)
            for ft in range(4):
                q_ps = mps.tile([128, 128], F32, tag="setup_ps")
                for kh in range(2):
                    nc.tensor.matmul(q_ps, lhsT=wq32[:, kh, ft, :],
                                     rhs=xnT32[:, kh, 4, :],
                                     start=(kh == 0), stop=(kh == 1))
                nc.scalar.copy(qT[:, ft, :], q_ps)
            # V with a ones-column appended per head: attn @ [v | 1] gives
            # av in cols 0:64 and the softmax denominator in col 64.
            v16e = cst.tile([128, 4, H, DH + 1], BF16)
            nc.vector.memset(v16e.rearrange("p a h d -> p (a h d)"), 1.0)
            NV = DH if KV == "nov16e" else DH + 1
            for st in range(4):
                v_ps = mps.tile([128, H * DH], F32, tag="setup_ps")
                for kh in range(2):
                    nc.tensor.matmul(v_ps,
                                     lhsT=xnT[:, kh, st, :],
                                     rhs=wv16[:, kh, :, :].rearrange("p a b -> p (a b)"),
                                     start=(kh == 0), stop=(kh == 1))
                if KV == "nov16e":
                    nc.scalar.copy(
                        v16e[:, st, :, 0:DH].rearrange("p h d -> p h d"),
                        v_ps.rearrange("p (h d) -> p h d", h=H))
                else:
                    nc.scalar.copy(v16e[:, st, :, 0:DH],
                                   v_ps.rearrange("p (h d) -> p h d", h=H))

            # av accumulates across blocks in SBUF (PSUM allows only one
            # pending accumulation group per bank, so long-lived per-head
            # PSUM chains are structurally unsound).
            avacc = cst.tile([128, H, DH + 2], F32)
            nc.vector.memset(avacc.rearrange("p h d -> p (h d)"), 0.0)
            # attn tiles: [j, t, h, i]; masked region stays 0 from this memset
            attn_all = cst.tile([128, 4, H, 128], BF16)
            nc.vector.memset(attn_all.rearrange("p a h d -> p (a h d)"), 0.0)

            # ------------- edges + attention, chunks ascending -------------

            def bcast8(ap2d, n):
                return ap2d.rearrange("p (c o) -> p c o", o=1).broadcast_to([128, n, H])

            def sim_block(t):
                n = NR[t]
                sl = slice(TOFF[t], TOFF[t + 1])
                # rinv = Exp(-0.5*Log(ms+eps)): Ln/Exp/Square share one ACT
                # table set
                srt = wk.tile([128, 128], F32, tag="srt")
                nc.scalar.activation(out=srt[:, 0:n], in_=ss_all[:, sl],
                                     func=AF.Ln, bias=epsc, scale=1.0 / DIM)
                rinv = wk.tile([128, 128], F32, tag="rinv")
                nc.scalar.activation(out=rinv[:, 0:n], in_=srt[:, 0:n],
                                     func=AF.Exp, scale=-0.5)
                stgtmp = wk.tile([128, 128, H], BF16, tag="stgtmp")
                nc.vector.tensor_mul(stgtmp[:, 0:n, :], stgraw[t],
                                     bcast8(rinv[:, 0:n], n))
                # stgfin stored h-major so the scr add is ONE op over the
                # whole sim tile -- a per-h DVE read would race the PE still
                # writing later heads in the same PSUM bank (fatal).
                stgfin = wk.tile([128, H, 128], BF16, tag="stgfin")
                if KV == "nostg":
                    for h in range(H):
                        nc.vector.tensor_add(stgfin[:, h, 0:n],
                                             stgtmp[:, 0:n, h],
                                             mcolh[:, sl, h])
                else:
                    nc.vector.tensor_add(
                        stgfin[:, :, 0:n].rearrange("p h n -> p n h"),
                        stgtmp[:, 0:n, :], mcolh[:, sl, :])
                if KV == "justtmp":
                    return
                sim_ps = sps.tile([128, H, 128], F32, tag="sim")
                for h in range(KSIM):
                    pb, ft = (h % 2) * 64, h // 2
                    nc.tensor.matmul(sim_ps[:, h, 0:n],
                                     lhsT=kT[pb:pb + 64, ft, t * 128:(t + 1) * 128],
                                     rhs=qT[pb:pb + 64, ft, 32 * t:128],
                                     start=True, stop=True)
                if KV == "justsim":
                    return
                scr = wk.tile([128, H, 128], F32, tag="scr")
                nc.vector.tensor_add(scr[:, :, 0:n], sim_ps[:, :, 0:n],
                                     stgfin[:, :, 0:n])
                nc.scalar.activation(out=attn_all[:, t, :, 32 * t:],
                                     in_=scr[:, :, 0:n], func=AF.Exp)
                if KV == "noav":
                    return
                avb = []
                for g in range(2):
                    avb.append(avps.tile([128, 4, DH + 2], F32, tag=f"av{g}",
                                         name=f"avb{g}_{t}"))
                for h in range(H):
                    nc.tensor.matmul(avb[h // 4][:, h % 4, 0:NV],
                                     lhsT=attn_all[:, t, h, :],
                                     rhs=v16e[:, t, h, 0:NV],
                                     start=True, stop=True)
                for g in range(2):
                    nc.vector.tensor_add(avacc[:, 4 * g:4 * g + 4, 0:DH + 1],
                                         avacc[:, 4 * g:4 * g + 4, 0:DH + 1],
                                         avb[g][:, :, 0:DH + 1])

            for c in range(NCH):
                tb = CHUNK_T[c]
                lo = 32 * c - TOFF[tb]          # local row offset in block tb
                ec = ep.tile([128, CH, DIM], FP8, tag="ec")
                nc.scalar.dma_start(out=ec, in_=ef8_ext[:, c, :, :])
                mode = SS_MODE[c]
                if mode == "a":
                    sq = sqp.tile([128, CH, DIM], BF16, tag="sqg")
                    nc.scalar.activation(out=sq, in_=ec, func=AF.Square)
                elif mode == "g":
                    sq = sqp.tile([128, CH, DIM], BF16, tag="sqg")
                    nc.gpsimd.tensor_tensor(sq, ec, ec, op=MUL)
                else:
                    sq = sqp.tile([128, CH, DIM], BF16, tag="sqg")
                    nc.vector.tensor_tensor(sq, ec, ec, op=MUL)
                bias_chunk = bps.tile([128, CH, 10], F32, tag="bias")
                for q in range(CH):
                    if KV != "noss":
                        for kh in range(2):
                            nc.tensor.matmul(bias_chunk[:, q, 8:9],
                                             lhsT=sq[:, q, kh * 128:(kh + 1) * 128],
                                             rhs=ones16,
                                             start=(kh == 0), stop=(kh == 1))
                    for kh in range(2):
                        nc.tensor.matmul(bias_chunk[:, q, 0:8],
                                         lhsT=ec[:, q, kh * 128:(kh + 1) * 128],
                                         rhs=we16[:, kh, :],
                                         start=(kh == 0), stop=(kh == 1))
                nc.vector.tensor_copy(stgraw[tb][:, lo:lo + CH, :],
                                      bias_chunk[:, :, 0:8])
                if KV != "noss":
                    nc.vector.tensor_copy(ss_all[:, 32 * c:32 * (c + 1)],
                                          bias_chunk[:, :, 8])
                if KV != "chunksonly":
                    if c == 3:
                        sim_block(0)
                    elif c == 6:
                        sim_block(1)
                    elif c == 8:
                        sim_block(2)
            if KV != "chunksonly":
                sim_block(3)

            # ---------------- epilogue ----------------
            rv = cst.tile([128, H], F32)
            if KV == "justsim":
                out_sb0 = cst.tile([128, DIM], F32)
                nc.vector.tensor_copy(out_sb0, ss_all[:, 0:DIM])
                nc.sync.dma_start(out=out_ext[:, :], in_=out_sb0)
            if KV == "justtmp":
                out_sb0 = cst.tile([128, DIM], F32)
                nc.vector.tensor_copy(out_sb0, ss_all[:, 0:DIM])
                nc.sync.dma_start(out=out_ext[:, :], in_=out_sb0)
            if KV == "noav":
                out_sb0 = cst.tile([128, DIM], F32)
                nc.vector.tensor_copy(
                    out_sb0, attn_all[:, 0, 0:2, :].rearrange("p a b -> p (a b)"))
                nc.sync.dma_start(out=out_ext[:, :], in_=out_sb0)
            if KV == "chunksonly":
                out_sb0 = cst.tile([128, DIM], F32)
                nc.vector.tensor_copy(out_sb0, stgraw[0][:, 0:32, :].rearrange("p a b -> p (a b)"))
                nc.sync.dma_start(out=out_ext[:, :], in_=out_sb0)
            if KV == "nov16e":
                nc.vector.memset(rv, 1.0)
            elif KV not in ("chunksonly", "noav", "justtmp", "justsim"):
                nc.vector.reciprocal(rv, avacc[:, :, DH])
            av_sb = cst.tile([128, H * DH], BF16)
            for h in (range(H) if KV not in ("chunksonly", "noav", "justtmp", "justsim") else []):
                nc.vector.tensor_scalar(out=av_sb[:, h * DH:(h + 1) * DH],
                                        in0=avacc[:, h, 0:DH],
                                        scalar1=rv[:, h:h + 1], scalar2=None,
                                        op0=MUL)
            if KV not in ("chunksonly", "noav", "justtmp", "justsim"):
                avT = cst.tile([128, 4, 128], BF16)
                nc.sync.dma_start(out=avT, in_=av_sb, transpose=True)
                out_ps = mps.tile([128, DIM], F32, tag="outp")
                for q4 in range(4):
                    nc.tensor.matmul(out_ps, lhsT=avT[:, q4, :],
                                     rhs=wo16[:, q4, :],
                                     start=(q4 == 0), stop=(q4 == 3))
                out_sb = cst.tile([128, DIM], F32)
                nc.vector.tensor_copy(out_sb, out_ps)
                nc.sync.dma_start(out=out_ext[:, :], in_=out_sb)
            if debug:
                nc.sync.dma_start(out=dbg_ss[:, :], in_=ss_all)
                for t in range(4):
                    raw32 = cst.tile([128, NR[t], H], F32, tag=f"r32_{t}",
                                     name=f"r32_{t}")
                    nc.vector.tensor_copy(raw32, stgraw[t])
                    nc.sync.dma_start(out=dbg_raw[t][:, :, :], in_=raw32)
                av32 = cst.tile([128, H * DH], F32)
                nc.vector.tensor_copy(av32, av_sb)
                nc.sync.dma_start(out=dbg_av[:, :], in_=av32)
                at32 = cst.tile([128, 4, H, 128], F32)
                nc.vector.tensor_copy(at32, attn_all)
                nc.sync.dma_start(out=dbg_attn[:, :, :, :], in_=at32)
    return nc


_NC_CACHE = [None]
LAST_RESULT = [None]


def _pack_core(edges_b8, x, b, l, mask, b_edge):
    """Per-core host packing: fp8 causal-prefix edges, d-on-partitions,
    t-major tile order; plus the mask/b_edge tensor mcolh."""
    E8 = edges_b8[l::4]                       # [128, 512, 256] fp8
    eT_all = np.empty((128, NT, DIM), dtype=F8)
    mcolh = np.empty((128, NT, H), np.float32)
    jj = np.arange(128)
    for t in range(4):
        nr = NR[t]
        blk = E8[32 * t:, 128 * t:128 * (t + 1), :]        # [nr, 128, 256]
        # [p, r, kh, j] <- blk[r, j, kh*128+p]
        eT_all[:, TOFF[t]:TOFF[t] + nr, :] = (
            blk.transpose(2, 0, 1).reshape(2, 128, nr, 128)
            .transpose(1, 2, 0, 3).reshape(128, nr, DIM))
        r = np.arange(32 * t, 128)
        valid = (128 * t + jj[:, None] <= 4 * r[None, :] + l) \
            & mask[b, 128 * t + jj][:, None]               # [128, nr]
        mcolh[:, TOFF[t]:TOFF[t] + nr, :] = np.where(
            valid[:, :, None], b_edge[None, None, :], NEG)
    ef8 = np.ascontiguousarray(eT_all.reshape(128, NCH, CH, DIM))
    xq = np.ascontiguousarray(x[b, l::4])
    return ef8, mcolh.astype(BF), xq


def kernel(x, mask, edges, gamma_x, W_qkv, gamma_e, W_edge, b_edge, W_out):
    x = np.asarray(x, np.float32)
    mask = np.asarray(mask)
    edges = np.asarray(edges, np.float32)
    gamma_x = np.asarray(gamma_x, np.float32)
    W_qkv = np.asarray(W_qkv, np.float32)
    gamma_e = np.asarray(gamma_e, np.float32)
    W_edge = np.asarray(W_edge, np.float32)
    b_edge = np.asarray(b_edge, np.float32)
    W_out = np.asarray(W_out, np.float32)

    wqkv_f = (gamma_x[:, None] * W_qkv).copy()
    wqkv_f[:, :H * DH] *= DH ** 0.5
    wq32 = np.ascontiguousarray(
        wqkv_f[:, :1024].reshape(2, 128, 8, 128).transpose(1, 0, 2, 3))
    wv16 = np.ascontiguousarray(
        wqkv_f[:, 1024:1536].reshape(2, 128, 4, 128).transpose(1, 0, 2, 3)
    ).astype(BF)
    wedge_f = gamma_e[:, None] * W_edge
    we16 = np.ascontiguousarray(
        wedge_f.reshape(2, 128, H).transpose(1, 0, 2)).astype(BF)
    wo16 = np.ascontiguousarray(
        W_out.reshape(4, 128, DIM).transpose(1, 0, 2)).astype(BF)
    ident = np.eye(128, dtype=BF)

    # |e| <= 15 so e^2 <= 225 stays under TRN fp8e4's 240 max-normal
    edges8 = np.clip(edges, -15.0, 15.0).astype(F8)

    in_maps = []
    for c in range(8):
        b, l = c // 4, c % 4
        ef8, mcolh, xq = _pack_core(edges8[b], x, b, l, mask, b_edge)
        in_maps.append({
            "ef8": ef8, "xb": x[b], "xq": xq,
            "wq32": wq32, "wv16": wv16, "we16": we16, "wo16": wo16,
            "mcolh": mcolh, "ident": ident,
        })

    if _NC_CACHE[0] is None:
        nc = build()
        if not os.environ.get("NOSPLIT"):
            split_waits(nc)
        _NC_CACHE[0] = nc
    res = run_bass_kernel_spmd(_NC_CACHE[0], in_maps, core_ids=list(range(8)))
    LAST_RESULT[0] = res

    out = np.zeros((B, N, DIM), np.float32)
    for c in range(8):
        b, l = c // 4, c % 4
        out[b, l::4] = res.results[c]["out"]
    return out
